# revision 13
# baseline (speedup 1.0000x reference)
"""Involution1d Trainium2 kernel.

Problem (hardcoded shapes): x [4, 256, 8192] f32, W1 [64, 256], b1 [64],
W2 [112, 64], b2 [112]; C=256, G=16 groups of 16 channels, K=7, pad=3.

  kern[(g,k), l] = (W2 @ W1) x + (W2 b1 + b2)     (both GEMMs folded)
  out[b, c, l] = sum_k kern[b, g(c)*7+k, l] * xpad[b, c, l+k]

Sharding: 8 shards = (batch b, L-half); each core handles [256, 4096]
outputs from a 3-halo pre-padded x slice [256, 4102].

Per-core device kernel:
  - PE: kern_rep[j][128, 512] = Wfold_j @ x per (c-half j//7, k j%7),
    where Wfold_j = W2rep_j @ W1 is host-precomputed with W2's rows
    replicated across the 16 channels of each group, so kern lands in
    PSUM already broadcast to per-channel layout. Contraction over
    C=256 as two 128-chunk matmuls accumulated in PSUM.
  - DVE: acc = sum_k (kern_k + b2'_k) * x[:, l0+k : l0+k+512] using
    scalar_tensor_tensor (bias fused as the per-partition scalar).

All inputs for partitions 0-127 / 128-255 are packed into one blob DMA
each so no matmul ever needs two semaphore waits (TRN2 LDW allows 1).
"""

import numpy as np
from contextlib import ExitStack

import concourse.bass as bass
import concourse.mybir as mybir
from concourse import tile
from concourse.bass_utils import run_bass_kernel_spmd

C, G, K, R = 256, 16, 7, 4
B, L = 4, 8192
CR = C // R  # 64
PAD = (K - 1) // 2  # 3
NCORES = 8
LC = L // 2  # per-shard length 4096
LT = 512  # l-tile
NJ = 2 * K  # 14 (c-half, k) pairs
XW = LC + 2 * PAD  # 4102
WFW = NJ * 128  # 1792
F32 = mybir.dt.float32

_prog_cache = {}


OXA = 0
OWFA = XW
OB2 = XW + WFW
OXB = OB2 + NJ
OWFB = OXB + XW
BLOBW = OWFB + WFW  # 11802


def _build_program():
    nc = bass.Bass()
    # single input blob: [xa | wfa | b2t | xb | wfb] -> one SWDGE load,
    # so the kernel uses 1 SWDGE sem + 4 HWDGE queues (stores) and the
    # tail drain stays within the NOP wait budget.
    inp = nc.declare_dram_parameter("inp", [128, BLOBW], F32, isOutput=False)
    ys = nc.declare_dram_parameter("ys", [C, LC], F32, isOutput=True)

    add, mult = mybir.AluOpType.add, mybir.AluOpType.mult

    with tile.TileContext(nc) as tc, ExitStack() as ctx:
        xpool = ctx.enter_context(tc.tile_pool(name="x", bufs=1))
        accp = ctx.enter_context(tc.tile_pool(name="acc", bufs=4))
        scr = ctx.enter_context(tc.tile_pool(name="scr", bufs=1))
        kps = ctx.enter_context(tc.tile_pool(name="kps", bufs=8, space="PSUM"))

        ta = xpool.tile([128, BLOBW], F32, tag="ta")
        nc.gpsimd.dma_start(ta[:], inp[:])
        xa, wfa = ta[:, OXA : OXA + XW], ta[:, OWFA : OWFA + WFW]
        b2s = ta[:, OB2 : OB2 + NJ]
        xb, wfb = ta[:, OXB : OXB + XW], ta[:, OWFB : OWFB + WFW]

        # warm-up: let DVE observe the input-blob DMA so later DVE ops
        # only ever need the single PE wait.
        s0 = scr.tile([128, 1], F32, tag="s0")
        nc.vector.tensor_copy(s0[:], ta[:, 0:1])

        # 4 output groups of [128, 2048] -> exactly 4 HWDGE out-DMAs on
        # distinct HW queues: no queue-reuse waits on the SP engine.
        for t in range(LC // (4 * LT)):
            for half in range(2):
                xh = xa if half == 0 else xb
                acc = accp.tile([128, 4 * LT], F32, tag="acc")
                for s in range(4):
                    l0 = (4 * t + s) * LT
                    av = acc[:, s * LT : (s + 1) * LT]
                    kts = []
                    for k in range(K):
                        j = half * K + k
                        kp = kps.tile([128, LT], F32, tag="kp")
                        nc.tensor.matmul(
                            kp[:], wfa[:, j * 128 : (j + 1) * 128],
                            xa[:, l0 + PAD : l0 + PAD + LT],
                            start=True, stop=False,
                        )
                        nc.tensor.matmul(
                            kp[:], wfb[:, j * 128 : (j + 1) * 128],
                            xb[:, l0 + PAD : l0 + PAD + LT],
                            start=False, stop=True,
                        )
                        kts.append(kp)
                    j0 = half * K
                    nc.vector.scalar_tensor_tensor(
                        av, kts[0][:], b2s[:, j0 : j0 + 1], xh[:, l0 : l0 + LT],
                        op0=add, op1=mult,
                    )
                    for k in range(1, K):
                        j = half * K + k
                        # in-place on the PSUM bank: kp <- (kp + b2) * x_shift
                        nc.vector.scalar_tensor_tensor(
                            kts[k][:], kts[k][:], b2s[:, j : j + 1],
                            xh[:, l0 + k : l0 + k + LT],
                            op0=add, op1=mult,
                        )
                        nc.vector.tensor_add(av, av, kts[k][:])
                nc.sync.dma_start(
                    ys[half * 128 : (half + 1) * 128, 4 * t * LT : 4 * (t + 1) * LT],
                    acc[:],
                )
    _split_multiwaits(nc)
    return nc


def _split_multiwaits(nc):
    """walrus on this toolchain accepts at most ONE sync wait per
    instruction. The only multi-wait instruction is Tile's kernel-tail
    drain (one wait per in-flight proc). Keep a single wait on the
    highest-value DMAHW sem = the last-issued store. That store's trigger
    already orders after the final DVE op (which orders after all PE
    work), the input load completed long before (everything consumed
    it), and the earlier same-size stores on independent queues complete
    before the later one."""
    for b in nc.m.functions[0].blocks:
        for inst in b.instructions:
            si = inst.sync_info
            if si is None or len(si.on_wait) <= 1:
                continue
            waits = list(si.on_wait)
            hw = [w for w in waits if "DMAHW" in (w.ant_name or "")]
            keep = max(hw, key=lambda w: (w.wait_value, w.ant_name)) if hw else waits[0]
            si.on_wait = [keep]
            inst.sync_info = si


def _host_prep(x, W1, b1, W2, b2):
    x = np.ascontiguousarray(np.asarray(x, dtype=np.float32))
    W1 = np.asarray(W1, dtype=np.float64)
    b1 = np.asarray(b1, dtype=np.float64)
    W2 = np.asarray(W2, dtype=np.float64)
    b2 = np.asarray(b2, dtype=np.float64)

    b2p = W2 @ b1 + b2  # [112]

    cidx = np.arange(C)
    rows0 = (cidx // (C // G)) * K  # W2 row for (g(c), k=0)
    wfold = np.zeros((C, WFW), dtype=np.float32)  # [256, 14*128] = Wfold_j^T stacked
    b2t = np.zeros((128, NJ), dtype=np.float32)
    WF = W2 @ W1  # [112, 256] fp64
    for half in range(2):
        sel = slice(half * 128, (half + 1) * 128)
        for k in range(K):
            j = half * K + k
            r = rows0[sel] + k  # [128] W2/WF row per channel
            wfold[:, j * 128 : (j + 1) * 128] = WF[r, :].T.astype(np.float32)
            b2t[:, j] = b2p[r].astype(np.float32)

    xp = np.pad(x, ((0, 0), (0, 0), (PAD, PAD)))  # [B, C, L + 6]
    in_maps = []
    for core in range(NCORES):
        b, half = divmod(core, 2)
        xsl = xp[b, :, half * LC : half * LC + XW]  # [256, 4102]
        blob = np.concatenate(
            [xsl[0:128], wfold[0:128], b2t, xsl[128:256], wfold[128:256]], axis=1
        )
        in_maps.append({"inp": np.ascontiguousarray(blob)})
    return in_maps


def kernel(x, W1, b1, W2, b2):
    in_maps = _host_prep(x, W1, b1, W2, b2)
    if "nc" not in _prog_cache:
        _prog_cache["nc"] = _build_program()
    nc = _prog_cache["nc"]
    res = run_bass_kernel_spmd(nc, in_maps, list(range(NCORES))).results
    out = np.empty((B, C, L), dtype=np.float32)
    for core in range(NCORES):
        b, half = divmod(core, 2)
        out[b, :, half * LC : (half + 1) * LC] = res[core]["ys"]
    return out


# revision 14
# speedup vs baseline: 1.1392x; 1.1392x over previous
"""Involution1d Trainium2 kernel.

Problem (hardcoded shapes): x [4, 256, 8192] f32, W1 [64, 256], b1 [64],
W2 [112, 64], b2 [112]; C=256, G=16 groups of 16 channels, K=7, pad=3.

  kern[(g,k), l] = (W2 @ W1) x + (W2 b1 + b2)     (both GEMMs folded)
  out[b, c, l] = sum_k kern[b, g(c)*7+k, l] * xpad[b, c, l+k]

Sharding: 8 shards = (batch b, L-half); each core handles [256, 4096]
outputs from a 3-halo pre-padded x slice [256, 4102].

Per-core device kernel:
  - PE: kern_rep[j][128, 512] = Wfold_j @ x per (c-half j//7, k j%7),
    where Wfold_j = W2rep_j @ W1 is host-precomputed with W2's rows
    replicated across the 16 channels of each group, so kern lands in
    PSUM already broadcast to per-channel layout. Contraction over
    C=256 as two 128-chunk matmuls accumulated in PSUM.
  - DVE: acc = sum_k (kern_k + b2'_k) * x[:, l0+k : l0+k+512] using
    scalar_tensor_tensor (bias fused as the per-partition scalar; the
    k>0 products computed in place on the kern PSUM bank).

This toolchain accepts at most ONE sync wait per instruction, which
drives the I/O structure: 4 input blobs (each one DMA/one tile so no
consumer ever waits two DMA sems; wf packed together with the x range
its matmuls read) on HW queues 0-3, 4 stores on virgin queues 4-7,
dummy PE/DVE ops pre-observe each blob, and the kernel-tail drain is
reduced to the single last-store wait (everything else provably done).
"""

import numpy as np
from contextlib import ExitStack

import concourse.bass as bass
import concourse.mybir as mybir
from concourse import tile
from concourse.bass_utils import run_bass_kernel_spmd

C, G, K, R = 256, 16, 7, 4
B, L = 4, 8192
CR = C // R  # 64
PAD = (K - 1) // 2  # 3
NCORES = 8
LC = L // 2  # per-shard length 4096
LT = 512  # l-tile
NJ = 2 * K  # 14 (c-half, k) pairs
XW = LC + 2 * PAD  # 4102
WFW = NJ * 128  # 1792
XLO = 4 * LT + 2 * PAD + 4  # 2058: x cols [0, 2058) for s=0..3
XHI = XW - (LC // 2)  # 2054: x cols [2048, 4102) for s=4..7
AW = WFW + NJ + XLO  # blob A: [wfa | b2t | xa_lo]
BW = WFW + XLO  # blob B: [wfb | xb_lo]
F32 = mybir.dt.float32

_prog_cache = {}


def _build_program():
    nc = bass.Bass()
    ba = nc.declare_dram_parameter("ba", [128, AW], F32, isOutput=False)
    ba2 = nc.declare_dram_parameter("ba2", [128, XHI], F32, isOutput=False)
    bb = nc.declare_dram_parameter("bb", [128, BW], F32, isOutput=False)
    bb2 = nc.declare_dram_parameter("bb2", [128, XHI], F32, isOutput=False)
    ys = nc.declare_dram_parameter("ys", [C, LC], F32, isOutput=True)

    add, mult = mybir.AluOpType.add, mybir.AluOpType.mult

    with tile.TileContext(nc) as tc, ExitStack() as ctx:
        xpool = ctx.enter_context(tc.tile_pool(name="x", bufs=1))
        accp = ctx.enter_context(tc.tile_pool(name="acc", bufs=4))
        scr = ctx.enter_context(tc.tile_pool(name="scr", bufs=4))
        kps = ctx.enter_context(tc.tile_pool(name="kps", bufs=7, space="PSUM"))
        dps = ctx.enter_context(tc.tile_pool(name="dps", bufs=1, space="PSUM"))

        ta = xpool.tile([128, AW], F32, tag="ta")
        nc.sync.dma_start(ta[:], ba[:])
        ta2 = xpool.tile([128, XHI], F32, tag="ta2")
        nc.sync.dma_start(ta2[:], ba2[:])
        tb = xpool.tile([128, BW], F32, tag="tb")
        nc.sync.dma_start(tb[:], bb[:])
        tb2 = xpool.tile([128, XHI], F32, tag="tb2")
        nc.sync.dma_start(tb2[:], bb2[:])

        wfa = ta[:, 0:WFW]
        b2s = ta[:, WFW : WFW + NJ]
        wfb = tb[:, 0:BW][:, 0:WFW]

        # DVE pre-observes each input blob (one wait per op) so the STT
        # chain below only ever needs its single PE wait.
        for i, t in enumerate((ta, ta2, tb, tb2)):
            s = scr.tile([128, 1], F32, tag=f"s{i}")
            nc.vector.tensor_copy(s[:], t[:, 0:1])
        # PE pre-observes the hi blobs via dummy 1-column matmuls.
        for i, t in enumerate((ta2, tb2)):
            d = dps.tile([1, 1], F32, tag="d")
            nc.tensor.matmul(d[:], t[:, 0:1], t[:, 0:1], start=True, stop=True)

        def xv(half, s, off):
            # view of x columns [s*512 + off, +512) for the given c-half
            base = s * LT + off
            if s < 4:
                t = ta if half == 0 else tb
                xoff = WFW + NJ if half == 0 else WFW
                return t[:, xoff + base : xoff + base + LT]
            t = ta2 if half == 0 else tb2
            return t[:, base - LC // 2 : base - LC // 2 + LT]

        # 4 output groups of [128, 2048] -> 4 HWDGE stores on queues 4-7
        # (loads took 0-3): no queue-reuse waits on the SP engine.
        for t in range(LC // (4 * LT)):
            for half in range(2):
                acc = accp.tile([128, 4 * LT], F32, tag="acc")
                for s4 in range(4):
                    s = 4 * t + s4
                    av = acc[:, s4 * LT : (s4 + 1) * LT]
                    kts = []
                    for k in range(K):
                        j = half * K + k
                        kp = kps.tile([128, LT], F32, tag="kp")
                        nc.tensor.matmul(
                            kp[:], wfa[:, j * 128 : (j + 1) * 128],
                            xv(0, s, PAD), start=True, stop=False,
                        )
                        nc.tensor.matmul(
                            kp[:], wfb[:, j * 128 : (j + 1) * 128],
                            xv(1, s, PAD), start=False, stop=True,
                        )
                        kts.append(kp)
                    j0 = half * K
                    nc.vector.scalar_tensor_tensor(
                        av, kts[0][:], b2s[:, j0 : j0 + 1], xv(half, s, 0),
                        op0=add, op1=mult,
                    )
                    for k in range(1, K):
                        j = half * K + k
                        # in-place on the PSUM bank: kp <- (kp + b2) * x_shift
                        nc.vector.scalar_tensor_tensor(
                            kts[k][:], kts[k][:], b2s[:, j : j + 1],
                            xv(half, s, k), op0=add, op1=mult,
                        )
                        nc.vector.tensor_add(av, av, kts[k][:])
                nc.sync.dma_start(
                    ys[half * 128 : (half + 1) * 128, 4 * t * LT : 4 * (t + 1) * LT],
                    acc[:],
                )
    _split_multiwaits(nc)
    return nc


def _split_multiwaits(nc):
    """walrus on this toolchain accepts at most ONE sync wait per
    instruction. The only multi-wait instruction is Tile's kernel-tail
    drain (one wait per in-flight proc). Keep a single wait on the
    highest DMAHW sem = the last-issued store. That store's trigger
    already orders after the final DVE op (which orders after all PE
    work), the input loads completed long before (all compute consumed
    them), and the earlier same-size stores on independent queues
    complete before the later one."""
    for b in nc.m.functions[0].blocks:
        for inst in b.instructions:
            si = inst.sync_info
            if si is None or len(si.on_wait) <= 1:
                continue
            waits = list(si.on_wait)
            hw = [w for w in waits if "DMAHW" in (w.ant_name or "")]
            keep = max(hw, key=lambda w: (w.wait_value, w.ant_name)) if hw else waits[0]
            si.on_wait = [keep]
            inst.sync_info = si


def _host_prep(x, W1, b1, W2, b2):
    x = np.ascontiguousarray(np.asarray(x, dtype=np.float32))
    W1 = np.asarray(W1, dtype=np.float64)
    b1 = np.asarray(b1, dtype=np.float64)
    W2 = np.asarray(W2, dtype=np.float64)
    b2 = np.asarray(b2, dtype=np.float64)

    b2p = W2 @ b1 + b2  # [112]

    cidx = np.arange(C)
    rows0 = (cidx // (C // G)) * K  # W2 row for (g(c), k=0)
    wfold = np.zeros((C, WFW), dtype=np.float32)  # [256, 14*128] = Wfold_j^T stacked
    b2t = np.zeros((128, NJ), dtype=np.float32)
    WF = W2 @ W1  # [112, 256] fp64
    for half in range(2):
        sel = slice(half * 128, (half + 1) * 128)
        for k in range(K):
            j = half * K + k
            r = rows0[sel] + k  # [128] W2/WF row per channel
            wfold[:, j * 128 : (j + 1) * 128] = WF[r, :].T.astype(np.float32)
            b2t[:, j] = b2p[r].astype(np.float32)

    xp = np.pad(x, ((0, 0), (0, 0), (PAD, PAD)))  # [B, C, L + 6]
    in_maps = []
    for core in range(NCORES):
        b, half = divmod(core, 2)
        xsl = xp[b, :, half * LC : half * LC + XW]  # [256, 4102]
        in_maps.append({
            "ba": np.ascontiguousarray(
                np.concatenate([wfold[0:128], b2t, xsl[0:128, 0:XLO]], axis=1)
            ),
            "ba2": np.ascontiguousarray(xsl[0:128, LC // 2 : XW]),
            "bb": np.ascontiguousarray(
                np.concatenate([wfold[128:256], xsl[128:256, 0:XLO]], axis=1)
            ),
            "bb2": np.ascontiguousarray(xsl[128:256, LC // 2 : XW]),
        })
    return in_maps


def kernel(x, W1, b1, W2, b2):
    in_maps = _host_prep(x, W1, b1, W2, b2)
    if "nc" not in _prog_cache:
        _prog_cache["nc"] = _build_program()
    nc = _prog_cache["nc"]
    res = run_bass_kernel_spmd(nc, in_maps, list(range(NCORES))).results
    out = np.empty((B, C, L), dtype=np.float32)
    for core in range(NCORES):
        b, half = divmod(core, 2)
        out[b, :, half * LC : (half + 1) * LC] = res[core]["ys"]
    return out
